# revision 69
# baseline (speedup 1.0000x reference)
"""Trainium2 Bass kernel for nn_BackwardStep_38749194944853.

Batched ADMM QP solve (OSQP-style), N=1024 independent QPs of dim nx=128 with
mi=128 inequality + me=32 doubled equality constraints, 100 fixed iterations.
Pure data-parallel over 8 cores, 128 QPs per core.

Phase A (per element, software-pipelined K_PIPE=4 generators so strict-FIFO
ACT/DVE queues interleave across elements):
  K = Q + (1+sigma) I + rho (Ai'Ai + 2 Ae'Ae)   (bf16 products, fp32 psum)
  Kinv via Newton-Schulz: bf16 state (4 steps incl. fused scalar init), then
  one split-bf16 polish X8 = 2X + X(negK X) with negK = negKb+negKlo and the
  psum product split g1hi+g1lo — exact bf16 products / fp32 accumulation
  square away the bf16 state floor without fp32 (4-cycle) matmuls.
  M = Kinv At' (bf16); G = At M; H = M^T via bf16 PE transposes, kept
  resident in SBUF (HallT + quad-stacked HallB) — no DRAM spill.
  Stationaries stored bf16, all shaped for 128-col FWL loads with 4 moving
  columns per quad instruction:
    T1    per-element -G[0:128,0:128]          (128 instr/half-iter, N=1)
    W4    quad M-packed -G[0:128,128:160]      (32 instr, N=4)
    G2eD  quad block-diagonal -G[128:160,128:160] (32 instr, N=4,
          accumulates garbage-free onto W4's psum rows via block-sparse rhs)
    G2A   quad K-stacked -G[128:160,0:128]     (32 instr, N=4)
Phase B (98 half-iters; For_i(49) x 2, in-place state):
  s' = C' + (0.5/rho)B + 0.5 s - G p~,  B = rho|s|, p~ = [B_i; B_e2-B_e3]
  Each half-iter is two independent 64-element halves; a half's DVE state
  chain overlaps the other half's PE block. All derived-state work
  (B_e/pbot/pbotD rebuild and the pending s_e update, via persistent
  u2g/he_sb) happens at body START where it hides under the t1top block,
  keeping the For_i end-of-body barrier drain minimal.
Final: x = M (rho uC - p~_99) + s_vec from SBUF-resident bf16 H (tops
per-element, bots as 32 quad matmuls with block-scattered f).

Progression (8 cores, NTFF HW exec): 7.27 ms baseline -> 3.49 (quad-packed
e-block matvec) -> 2.55 (phase A pipelining) -> 2.07 (bf16 paths, SBUF H)
-> 1.93 (body-start state tails) -> 1.77 (split-bf16 polish, quad final
matvec, cross-body s_e update) -> 1.66 ms (all-bf16 phase A: bf16 s/d
column matvecs, fused psum->bf16 casts; no fp32 PE ops remain per element)
-> 1.65 ms (Bib bf16 Abs issued ahead of fp32 B_i, shortening the
inter-iteration critical ACT chain). Rel err 5.2e-3 vs fp64 reference
replica (gate 2e-2).
"""
import os
from collections import deque

import numpy as np

import concourse.bass as bass
import concourse.bacc as bacc
import concourse.mybir as mybir
from concourse.tile import TileContext
from concourse.masks import make_identity
from concourse.bass_utils import run_bass_kernel_spmd

F32 = mybir.dt.float32
BF16 = mybir.dt.bfloat16
ALU = mybir.AluOpType
AFT = mybir.ActivationFunctionType

NCORES = 8
P = 128            # elements per core
NX = 128           # QP dimension
MI = 128           # inequality rows
ME = 32            # equality rows
MT = MI + ME       # 160 collapsed constraint dim

RHO = 0.1
EPS_ = 1e-4
ACOEF = 1.0 + 1e-6          # alpha + sigma added to Q's diagonal
C0 = float(2.0 / (1.1 + 7.5))  # Newton-Schulz scalar init
NS_LOOP = 5                  # NS iterations after the fused first one (6 total)
N_ITER = 100                 # reference ADMM iterations
N_AUPD = N_ITER - 2          # 98 a-state updates (a_1 given, w from a_99)
N_BODY = N_AUPD // 2         # 49 For_i bodies x 2 updates


def _col(t, n):
    return t[:, n:n + 1]


def _strided_cols(t, start, step, count, part=None):
    base = t[:, 0:1] if part is None else t[part[0]:part[1], 0:1]
    return bass.AP(tensor=base.tensor, offset=base.offset + start,
                   ap=[base.ap[0], [step, count]])


def build(n_el=P, n_body=N_BODY, ns_loop=NS_LOOP, taps=False):
    nc = bacc.Bacc()

    x_d = nc.dram_tensor("x", [P, NX, 1], F32, kind="ExternalInput")
    Q_d = nc.dram_tensor("Q", [P, NX, NX], F32, kind="ExternalInput")
    q_d = nc.dram_tensor("q", [P, NX, 1], F32, kind="ExternalInput")
    Ai_d = nc.dram_tensor("A_ineq", [P, MI, NX], F32, kind="ExternalInput")
    bi_d = nc.dram_tensor("b_ineq", [P, MI, 1], F32, kind="ExternalInput")
    Ae_d = nc.dram_tensor("A_eq", [P, ME, NX], F32, kind="ExternalInput")
    be_d = nc.dram_tensor("b_eq", [P, ME, 1], F32, kind="ExternalInput")
    out_d = nc.dram_tensor("out", [P, NX, 1], F32, kind="ExternalOutput")
    if taps:
        dbg_d = nc.dram_tensor("dbg", [8, 128, 256], F32, kind="ExternalOutput")

    with TileContext(nc) as tc:
        with (
            tc.tile_pool(name="consts", bufs=1) as consts,
            tc.tile_pool(name="gpool", bufs=1) as gpool,
            tc.tile_pool(name="work", bufs=5) as work,
            tc.tile_pool(name="wks", bufs=2) as wks,
            tc.tile_pool(name="pspool", bufs=8, space="PSUM") as pspool,
        ):
            # ---------------- constants ----------------
            ident = consts.tile([128, 128], F32)
            make_identity(nc, ident)
            negI = consts.tile([128, 128], F32)
            nc.vector.tensor_scalar_mul(negI, ident, -1.0)
            cIdent = consts.tile([128, 128], F32)
            nc.vector.tensor_scalar_mul(cIdent, ident, ACOEF)
            xinitI = consts.tile([128, 128], F32)
            nc.vector.tensor_scalar_mul(xinitI, ident, 2.0 * C0 - C0 * C0 * ACOEF)
            identb = consts.tile([128, 128], BF16)
            nc.vector.tensor_copy(identb, ident)

            # ---------------- persistent big tiles ----------------
            Q = n_el // 4  # quads: element n = 4q+k at partition block 32k
            # T1_all: per element -G[0:128, 0:128] bf16, [128, n_el*128]
            T1_all = gpool.tile([128, n_el * 128], BF16)
            # G2A_all: quad-stacked -G[128:160, 0:128] chunks: element 4q+k at
            # partitions 32k, cols q*128..; used as one [128,128] lhsT per quad
            # with a block-sparse rhs.
            G2A_all = gpool.tile([128, Q * 128], BF16)
            # W4_all: quad-packed -G[0:128, 128:160] blocks: element 4q+k in
            # cols q*128+32k.. (M-packed); one [128,128] FWL lhsT per quad,
            # dense rhs cols, element k's result in psum rows 32k.
            W4_all = gpool.tile([128, Q * 128], BF16)
            # G2eD_all: quad block-diagonal -G[128:160, 128:160] blocks:
            # element 4q+k at rows 32k, cols q*128+32k..; zeros elsewhere, so
            # a block-sparse rhs accumulates garbage-free onto W4's rows.
            G2eD_all = gpool.tile([128, Q * 128], BF16)
            # HallT/HallB: per element H = At Kinv, bf16, resident in SBUF
            # for the final matvec. Tops at cols n*128; bots quad-stacked
            # like G2A (element 4q+k at partitions 32k, cols q*128) so the
            # final bot matvec runs as 32 quad matmuls.
            HallT = gpool.tile([128, n_el * 128], BF16)
            HallB = gpool.tile([128, Q * 128], BF16)

            def t1(n):
                return T1_all[:, n * 128:(n + 1) * 128]

            def w4(n):
                a, g = n % 4, n // 4
                return W4_all[:, g * 128 + 32 * a:g * 128 + 32 * a + 32]

            def g2ed(n):
                a, g = n % 4, n // 4
                return G2eD_all[32 * a:32 * a + 32,
                                g * 128 + 32 * a:g * 128 + 32 * a + 32]

            # batched constants (m-layout: [m-part, element-cols])
            u_i = gpool.tile([128, n_el], F32)
            be_t = gpool.tile([32, n_el], F32)
            u_e2 = gpool.tile([32, n_el], F32)
            ruC_top = gpool.tile([128, n_el], F32)
            ruC_bot = gpool.tile([32, n_el], F32)
            nruC_top = gpool.tile([128, n_el], BF16)
            nruC_bot = gpool.tile([128, n_el], BF16)   # replicated x4
            nruC_botD = gpool.tile([128, n_el], BF16)  # block-sparse diag scatter
            nqv_all = gpool.tile([128, n_el], F32)
            nqvb_all = gpool.tile([128, n_el], BF16)
            Cp_i = gpool.tile([128, n_el], F32)
            Cp_e = gpool.tile([32, 2 * n_el], F32)     # [Cp_e2 | Cp_e3]
            S_all = gpool.tile([128, n_el], F32)
            u2g = gpool.tile([32, 2 * n_el], F32)
            D_all = gpool.tile([128, 2 * n_el], F32)   # [d_top | d_bot(32p)]
            # ADMM state (ping-pong a/b)
            s_i = [gpool.tile([128, n_el], F32, name=f"s_i{j}") for j in range(1)]
            s_e = [gpool.tile([32, 2 * n_el], F32, name=f"s_e{j}") for j in range(1)]
            B_i = [gpool.tile([128, n_el], F32, name=f"B_i{j}") for j in range(1)]
            B_e = [gpool.tile([32, 2 * n_el], F32, name=f"B_e{j}") for j in range(1)]
            Bib = [gpool.tile([128, n_el], BF16, name=f"Bib{j}") for j in range(1)]
            pbot = [gpool.tile([128, n_el], BF16, name=f"pbot{j}") for j in range(1)]
            pbotD = [gpool.tile([128, n_el], BF16, name=f"pbotD{j}") for j in range(1)]
            he_sb = [gpool.tile([32, n_el], F32, name=f"he_sb{j}") for j in range(1)]
            f_top = gpool.tile([128, n_el], F32)
            f_bot = gpool.tile([32, n_el], F32)
            xo = gpool.tile([128, n_el], F32)
            xout = gpool.tile([n_el, 128], F32)

            nc.vector.memset(pbotD[0], 0.0)
            nc.vector.memset(nruC_botD, 0.0)
            nc.vector.memset(G2eD_all, 0.0)

            # ---------------- batched input prep ----------------
            x_el = wks.tile([P, NX], F32, tag="xel")
            q_el = wks.tile([P, NX], F32, tag="qel")
            nc.sync.dma_start(out=x_el, in_=x_d[:, :, 0])
            nc.sync.dma_start(out=q_el, in_=q_d[:, :, 0])
            nq_el = wks.tile([P, NX], F32, tag="nqel")
            nc.vector.tensor_tensor(nq_el, x_el, q_el, ALU.subtract)  # -(q - x)
            nqps = pspool.tile([128, P], F32, tag="ps")
            nc.tensor.transpose(nqps, nq_el, ident)
            nc.vector.tensor_copy(nqv_all, nqps[:, 0:n_el])
            nc.vector.tensor_copy(nqvb_all, nqps[:, 0:n_el])

            bi_el = wks.tile([P, MI], F32, tag="biel")
            nc.sync.dma_start(out=bi_el, in_=bi_d[:, :, 0])
            bips = pspool.tile([128, P], F32, tag="ps")
            nc.tensor.transpose(bips, bi_el, ident)
            nc.vector.tensor_copy(u_i, bips[:, 0:n_el])

            be_el = wks.tile([P, ME], F32, tag="beel")
            nc.sync.dma_start(out=be_el, in_=be_d[:, :, 0])
            beps = pspool.tile([32, P], F32, tag="ps")
            nc.tensor.transpose(beps, be_el, ident)
            nc.vector.tensor_copy(be_t, beps[:, 0:n_el])

            nc.vector.tensor_scalar_add(u_e2, be_t, EPS_)
            nc.vector.tensor_scalar_mul(ruC_top, u_i, RHO)
            nc.vector.tensor_scalar(out=ruC_bot, in0=be_t, scalar1=2.0 * RHO,
                                    scalar2=RHO * EPS_, op0=ALU.mult, op1=ALU.add)
            nc.vector.tensor_scalar_mul(nruC_top, u_i, -RHO)
            nc.vector.tensor_scalar(out=nruC_bot[0:32, :], in0=be_t,
                                    scalar1=-2.0 * RHO, scalar2=-RHO * EPS_,
                                    op0=ALU.mult, op1=ALU.add)
            nc.vector.tensor_copy(nruC_bot[32:64, :], nruC_bot[0:32, :])
            nc.vector.tensor_copy(nruC_bot[64:128, :], nruC_bot[0:64, :])
            for k in range(4):
                nc.vector.tensor_copy(
                    _strided_cols(nruC_botD, k, 4, Q, part=(32 * k, 32 * k + 32)),
                    _strided_cols(nruC_bot, k, 4, Q, part=(32 * k, 32 * k + 32)))

            # ---------------- phase A: per-element factorization ----------------
            # Emitted as a K_PIPE-way software pipeline: each element's chain
            # is a staged generator and stages of neighbouring elements are
            # interleaved in issue order, so the strict-FIFO ACT/DVE queues
            # overlap work across elements instead of head-of-line blocking.
            SQ2 = float(np.sqrt(2.0))

            def elem_stages(n):
                a_, q_ = n % 4, n // 4
                Qt = work.tile([128, 128], F32, tag="Q")
                nc.sync.dma_start(out=Qt, in_=Q_d[n])
                Ait = work.tile([128, 128], F32, tag="Ai")
                nc.sync.dma_start(out=Ait, in_=Ai_d[n])
                Aet = work.tile([32, 128], F32, tag="Ae")
                nc.sync.dma_start(out=Aet, in_=Ae_d[n])
                yield

                Aib = work.tile([128, 128], BF16, tag="Aib")
                nc.vector.tensor_copy(Aib, Ait)
                Aeb = work.tile([32, 128], BF16, tag="Aeb")
                nc.scalar.activation(Aeb, Aet, AFT.Copy)
                Ae2 = work.tile([32, 128], BF16, tag="Ae2")
                nc.vector.tensor_scalar_mul(Ae2, Aet, SQ2)
                at_ps = pspool.tile([128, MT], BF16, tag="ps")
                nc.tensor.transpose(at_ps[:, 0:128], Aib, identb)
                nc.tensor.transpose(at_ps[:, 128:160], Aeb, identb[0:32, 0:32])
                ATb = work.tile([128, MT], BF16, tag="ATb")
                nc.scalar.activation(ATb, at_ps, AFT.Copy)
                yield

                # K = Q + (alpha+sigma) I + rho (Ai'Ai + 2 Ae'Ae); the rho
                # factor is folded into the psum consumer so only unscaled
                # bf16 casts of Ai / sqrt(2) Ae are needed.
                K_ps = pspool.tile([128, 128], F32, tag="ps")
                nc.tensor.matmul(K_ps, Aib, Aib, start=True, stop=False)
                nc.tensor.matmul(K_ps, Ae2, Ae2, start=False, stop=True)
                tmp = work.tile([128, 128], F32, tag="tmp")
                nc.vector.scalar_tensor_tensor(out=tmp, in0=K_ps, scalar=-RHO,
                                               in1=Qt, op0=ALU.mult,
                                               op1=ALU.subtract)
                negK = work.tile([128, 128], F32, tag="negK")
                nc.vector.scalar_tensor_tensor(out=negK, in0=tmp, scalar=1.0,
                                               in1=cIdent, op0=ALU.mult,
                                               op1=ALU.subtract)
                negKb = work.tile([128, 128], BF16, tag="negKb")
                nc.vector.tensor_copy(negKb, negK)
                negKlo = work.tile([128, 128], BF16, tag="negKlo")
                nc.vector.tensor_tensor(negKlo, negK, negKb, ALU.subtract)
                # NS state lives in bf16 (operand precision); the final fp32
                # polish squares away the bf16 state floor.
                Xb = work.tile([128, 128], BF16, tag="Xb", bufs=8)
                nc.vector.scalar_tensor_tensor(out=Xb, in0=tmp, scalar=C0 * C0,
                                               in1=xinitI, op0=ALU.mult,
                                               op1=ALU.add)
                yield

                for k in range(ns_loop - 2):
                    G1_ps = pspool.tile([128, 128], F32, tag="ps")
                    nc.tensor.matmul(G1_ps, negKb, Xb, start=True, stop=True)
                    g1 = work.tile([128, 128], BF16, tag="g1", bufs=8)
                    nc.scalar.activation(g1, G1_ps, AFT.Copy)
                    X2_ps = pspool.tile([128, 128], F32, tag="ps")
                    nc.tensor.matmul(X2_ps, Xb, g1, start=True, stop=True)
                    Xn = work.tile([128, 128], BF16, tag="Xb", bufs=8)
                    nc.vector.scalar_tensor_tensor(out=Xn, in0=Xb, scalar=2.0,
                                                   in1=X2_ps, op0=ALU.mult,
                                                   op1=ALU.add)
                    Xb = Xn
                    yield

                # Polish iteration at fp32 accuracy from bf16 operand pairs:
                # X8 = 2 Xb + Xb (negK Xb) with negK = negKb + negKlo and the
                # psum product split g1hi + g1lo — exact bf16 products, fp32
                # accumulation, so the bf16 state floor is squared away.
                # Explicit transpose feeds lhsT (Xb is not exactly symmetric).
                XbT_ps = pspool.tile([128, 128], BF16, tag="ps")
                nc.tensor.transpose(XbT_ps, Xb, identb)
                XbT = work.tile([128, 128], BF16, tag="XbT")
                nc.scalar.activation(XbT, XbT_ps, AFT.Copy)
                G1p = pspool.tile([128, 128], F32, tag="ps")
                nc.tensor.matmul(G1p, negKb, Xb, start=True, stop=False)
                nc.tensor.matmul(G1p, negKlo, Xb, start=False, stop=True)
                g1hi = work.tile([128, 128], BF16, tag="g1hi")
                nc.scalar.activation(g1hi, G1p, AFT.Copy)
                g1lo = work.tile([128, 128], BF16, tag="g1lo")
                nc.vector.tensor_tensor(g1lo, G1p, g1hi, ALU.subtract)
                X2p = pspool.tile([128, 128], F32, tag="ps")
                nc.tensor.matmul(X2p, XbT, g1hi, start=True, stop=False)
                nc.tensor.matmul(X2p, XbT, g1lo, start=False, stop=True)
                Xb8 = work.tile([128, 128], BF16, tag="Xb8")
                nc.vector.scalar_tensor_tensor(out=Xb8, in0=Xb, scalar=2.0,
                                               in1=X2p, op0=ALU.mult,
                                               op1=ALU.add)
                yield

                # M = Kinv At' (bf16 operands; fp32 accumulation in PSUM)
                Ms_ps = pspool.tile([128, MT], F32, tag="ps")
                nc.tensor.matmul(Ms_ps, Xb8, ATb, start=True, stop=True)
                Msb = work.tile([128, MT], BF16, tag="Msb")
                nc.vector.tensor_copy(Msb, Ms_ps)
                yield

                # H = At Kinv = Ms^T via bf16 PE transposes; spilled bf16.
                # s_vec / d columns go to a small fp32 psum.
                H_ps = pspool.tile([128, 256], BF16, tag="ps")
                nc.tensor.transpose(H_ps[:, 0:128], Msb[:, 0:128], identb)
                nc.tensor.transpose(H_ps[0:32, 128:256], Msb[:, 128:160],
                                    identb)
                sd_ps = pspool.tile([128, 4], F32, tag="ps")
                nc.tensor.matmul(sd_ps[:, 0:1], Xb8, _col(nqvb_all, n),
                                 start=True, stop=True, skip_group_check=True)
                nc.tensor.matmul(sd_ps[:, 1:2], Msb[:, 0:128],
                                 _col(nqvb_all, n),
                                 start=True, stop=True, skip_group_check=True)
                nc.tensor.matmul(sd_ps[0:32, 2:3], Msb[:, 128:160],
                                 _col(nqvb_all, n),
                                 start=True, stop=True, skip_group_check=True)
                Gr1_ps = pspool.tile([128, MT], F32, tag="ps")
                nc.tensor.matmul(Gr1_ps, ATb[:, 0:128], Msb, start=True,
                                 stop=True)
                Gr2_ps = pspool.tile([32, MT], F32, tag="ps")
                nc.tensor.matmul(Gr2_ps, ATb[:, 128:160], Msb, start=True,
                                 stop=True)
                yield

                nc.vector.tensor_copy(_col(S_all, n), sd_ps[:, 0:1])
                nc.vector.tensor_copy(_col(D_all, n), sd_ps[:, 1:2])
                nc.vector.tensor_copy(D_all[0:32, n_el + n:n_el + n + 1],
                                      sd_ps[0:32, 2:3])
                nc.vector.tensor_scalar_mul(t1(n), Gr1_ps[:, 0:128], -1.0)
                nc.vector.tensor_scalar_mul(w4(n), Gr1_ps[:, 128:160], -1.0)
                nc.vector.tensor_scalar_mul(
                    G2A_all[32 * a_:32 * a_ + 32, q_ * 128:(q_ + 1) * 128],
                    Gr2_ps[:, 0:128], -1.0)
                nc.vector.tensor_scalar_mul(g2ed(n), Gr2_ps[:, 128:160], -1.0)
                nc.scalar.activation(HallT[:, n * 128:(n + 1) * 128],
                                     H_ps[:, 0:128], AFT.Copy)
                nc.scalar.activation(
                    HallB[32 * a_:32 * a_ + 32, q_ * 128:(q_ + 1) * 128],
                    H_ps[0:32, 128:256], AFT.Copy)

                if taps and n == 0:
                    nc.sync.dma_start(out=dbg_d[0, :, 0:128], in_=negK)
                    nc.sync.dma_start(out=dbg_d[1, :, 0:128], in_=Xb8)
                    nc.sync.dma_start(out=dbg_d[2, :, 0:MT], in_=Msb)
                    nc.sync.dma_start(out=dbg_d[3, :, 0:128], in_=negK)

            K_PIPE = 4
            pend = deque()
            nextn = 0
            while pend or nextn < n_el:
                while len(pend) < K_PIPE and nextn < n_el:
                    pend.append(elem_stages(nextn))
                    nextn += 1
                g = pend.popleft()
                try:
                    next(g)
                    pend.append(g)
                except StopIteration:
                    pass

            # ---------------- s1 init + C' prepass ----------------
            # top: psum = d_i - u_i (s1), then + g0_i (C')
            S1T = pspool.tile([128, n_el], F32, tag="ps")
            nc.tensor.matmul(S1T, negI, u_i, start=True, stop=False,
                             skip_group_check=True)
            nc.tensor.matmul(S1T, ident, D_all[:, 0:n_el], start=False, stop=False,
                             skip_group_check=True)
            nc.vector.tensor_copy(s_i[0], S1T)
            S1E = pspool.tile([32, n_el], F32, tag="ps")
            nc.tensor.matmul(S1E, negI[0:32, 0:32], u_e2, start=True, stop=False,
                             skip_group_check=True)
            nc.tensor.matmul(S1E, ident[0:32, 0:32],
                             D_all[0:32, n_el:2 * n_el], start=False, stop=True,
                             skip_group_check=True)
            nc.vector.tensor_copy(s_e[0][:, 0:n_el], S1E)
            nc.vector.tensor_scalar(out=s_e[0][:, n_el:2 * n_el], in0=S1E,
                                    scalar1=-1.0, scalar2=-EPS_,
                                    op0=ALU.mult, op1=ALU.add)

            # accumulate g0 terms (bf16 G x bf16 -rho*uC) into the psums;
            # the e-block terms go through the quad-packed scratch.
            scr0 = pspool.tile([128, n_el], F32, tag="ps")
            for n in range(n_el):
                nc.tensor.matmul(_col(S1T, n), t1(n), _col(nruC_top, n),
                                 start=False, stop=False, skip_group_check=True)
            for q in range(Q):
                nc.tensor.matmul(S1T[:, 4 * q:4 * q + 4],
                                 G2A_all[:, q * 128:(q + 1) * 128],
                                 nruC_botD[:, 4 * q:4 * q + 4],
                                 start=False, stop=(q == Q - 1),
                                 skip_group_check=True)
            for q in range(Q):
                nc.tensor.matmul(scr0[:, 4 * q:4 * q + 4],
                                 W4_all[:, q * 128:(q + 1) * 128],
                                 nruC_top[:, 4 * q:4 * q + 4],
                                 start=(q == 0), stop=False,
                                 skip_group_check=True)
            for q in range(Q):
                nc.tensor.matmul(scr0[:, 4 * q:4 * q + 4],
                                 G2eD_all[:, q * 128:(q + 1) * 128],
                                 nruC_botD[:, 4 * q:4 * q + 4],
                                 start=False, stop=(q == Q - 1),
                                 skip_group_check=True)
            nc.vector.tensor_copy(Cp_i, S1T)
            for a in range(4):
                nc.vector.tensor_copy(
                    _strided_cols(he_sb[0], a, 4, Q),
                    _strided_cols(scr0, a, 4, Q, part=(32 * a, 32 * a + 32)))
            nc.vector.tensor_tensor(Cp_e[:, 0:n_el], s_e[0][:, 0:n_el],
                                    he_sb[0], ALU.add)
            nc.vector.tensor_scalar(out=Cp_e[:, n_el:2 * n_el],
                                    in0=Cp_e[:, 0:n_el],
                                    scalar1=-1.0, scalar2=-EPS_,
                                    op0=ALU.mult, op1=ALU.add)
            if taps:
                nc.sync.dma_start(out=dbg_d[5, :, 0:n_el], in_=Cp_i)
                nc.sync.dma_start(out=dbg_d[6, :, 0:n_el], in_=s_i[0])

            # ---------------- phase B: ADMM loop ----------------
            # Each half_iter is split into two independent 64-element halves;
            # half h's DVE state-update chain overlaps the other half's PE
            # block, so steady state is pure PE (weight-load bound).  B/Bib/
            # pbot/pbotD for a state index are computed as soon as that state
            # is produced (epilogue of the producing half), so the next PE
            # block never waits on DVE.
            HQ = Q // 2  # quads per half

            def state_tail(dst, h):
                """B / Bib for the i-block of half h of state dst."""
                hs = slice(64 * h, 64 * h + 64)
                # Bib first: it gates the next PE block; B_i (fp32, for the
                # exact relu path) is not needed until well into the next
                # half_iter, so it trails off the critical chain.
                nc.scalar.activation(Bib[dst][:, hs], s_i[dst][:, hs],
                                     AFT.Abs, scale=RHO)
                nc.scalar.activation(B_i[dst][:, hs], s_i[dst][:, hs],
                                     AFT.Abs, scale=RHO)

            def state_tail_e(dst):
                """B_e / pbot / pbotD for state dst (full width)."""
                nc.scalar.activation(B_e[dst], s_e[dst], AFT.Abs, scale=RHO)
                nc.vector.tensor_tensor(pbot[dst][0:32, :],
                                        B_e[dst][:, 0:n_el],
                                        B_e[dst][:, n_el:2 * n_el],
                                        ALU.subtract)
                for k in range(4):
                    nc.vector.tensor_copy(
                        _strided_cols(pbotD[dst], k, 4, Q,
                                      part=(32 * k, 32 * k + 32)),
                        _strided_cols(pbot[dst], k, 4, Q,
                                      part=(0, 32)))

            def half_iter(src, dst):
                # The pending e-state update from the previous half_iter is
                # applied at body start (u2g/he_sb persist across the For_i
                # back-edge), then the derived e-state is rebuilt — all of it
                # overlaps the first t1top PE block, keeping the end-of-body
                # drain (For_i barrier) short.
                nc.vector.tensor_tensor(s_e[src][:, 0:n_el],
                                        u2g[:, 0:n_el], he_sb[src], ALU.add)
                nc.vector.tensor_tensor(s_e[src][:, n_el:2 * n_el],
                                        u2g[:, n_el:2 * n_el],
                                        he_sb[src], ALU.subtract)
                state_tail_e(src)
                u1 = wks.tile([32, 2 * n_el], F32, tag="u1")
                nc.vector.scalar_tensor_tensor(out=u1, in0=B_e[src],
                                               scalar=0.5 / RHO, in1=Cp_e,
                                               op0=ALU.mult, op1=ALU.add)
                nc.vector.scalar_tensor_tensor(out=u2g, in0=s_e[src],
                                               scalar=0.5, in1=u1,
                                               op0=ALU.mult, op1=ALU.add)
                banks = []
                for h in (0, 1):
                    hs = slice(64 * h, 64 * h + 64)
                    bankT = pspool.tile([128, 64], F32, tag="ps")
                    scr = pspool.tile([128, 64], F32, tag="ps")
                    banks.append((bankT, scr))
                    for j in range(64):
                        n = 64 * h + j
                        nc.tensor.matmul(_col(bankT, j), t1(n),
                                         _col(Bib[src], n), start=(j == 0),
                                         stop=False, skip_group_check=True)
                    for i in range(HQ):
                        q = HQ * h + i
                        nc.tensor.matmul(bankT[:, 4 * i:4 * i + 4],
                                         G2A_all[:, q * 128:(q + 1) * 128],
                                         pbotD[src][:, 4 * q:4 * q + 4],
                                         start=False, stop=(i == HQ - 1),
                                         skip_group_check=True)
                    for i in range(HQ):
                        q = HQ * h + i
                        nc.tensor.matmul(scr[:, 4 * i:4 * i + 4],
                                         W4_all[:, q * 128:(q + 1) * 128],
                                         Bib[src][:, 4 * q:4 * q + 4],
                                         start=(i == 0), stop=False,
                                         skip_group_check=True)
                    for i in range(HQ):
                        q = HQ * h + i
                        nc.tensor.matmul(scr[:, 4 * i:4 * i + 4],
                                         G2eD_all[:, q * 128:(q + 1) * 128],
                                         pbotD[src][:, 4 * q:4 * q + 4],
                                         start=False, stop=(i == HQ - 1),
                                         skip_group_check=True)
                    # s_i' chain for half h (overlaps the other half's PE)
                    t1x = wks.tile([128, 64], F32, tag=f"t1x{h}")
                    nc.vector.scalar_tensor_tensor(out=t1x,
                                                   in0=B_i[src][:, hs],
                                                   scalar=0.5 / RHO,
                                                   in1=Cp_i[:, hs],
                                                   op0=ALU.mult, op1=ALU.add)
                    t2x = wks.tile([128, 64], F32, tag=f"t2x{h}")
                    nc.vector.scalar_tensor_tensor(out=t2x,
                                                   in0=s_i[src][:, hs],
                                                   scalar=0.5, in1=bankT,
                                                   op0=ALU.mult, op1=ALU.add)
                    nc.vector.tensor_tensor(s_i[dst][:, hs], t1x, t2x,
                                            ALU.add)
                    state_tail(dst, h)
                    # he gather for half h from its scratch psum
                    for a in range(4):
                        nc.vector.tensor_copy(
                            _strided_cols(he_sb[src], 64 * h + a, 4, HQ),
                            _strided_cols(scr, a, 4, HQ,
                                          part=(32 * a, 32 * a + 32)))

            # prologue: derived i-state for s1; e-update pipeline primed so
            # body 0's pending update is the identity (u2g = s1_e, he = 0)
            state_tail(0, 0)
            state_tail(0, 1)
            nc.vector.tensor_copy(u2g, s_e[0])
            nc.vector.memset(he_sb[0], 0.0)
            if n_body > 0:
                with tc.For_i(0, n_body, 1,
                              hint_engines=(mybir.EngineType.PE,)):
                    half_iter(0, 0)
                    half_iter(0, 0)

            # ---------------- final: x = M (rho uC - p~_99) + s_vec -------------
            # apply the last pending e-update, then B_e of the final state
            nc.vector.tensor_tensor(s_e[0][:, 0:n_el],
                                    u2g[:, 0:n_el], he_sb[0], ALU.add)
            nc.vector.tensor_tensor(s_e[0][:, n_el:2 * n_el],
                                    u2g[:, n_el:2 * n_el],
                                    he_sb[0], ALU.subtract)
            nc.scalar.activation(B_e[0], s_e[0], AFT.Abs, scale=RHO)
            nc.vector.tensor_tensor(f_bot, B_e[0][:, 0:n_el],
                                    B_e[0][:, n_el:2 * n_el], ALU.subtract)
            nc.vector.tensor_tensor(f_bot, ruC_bot, f_bot, ALU.subtract)
            nc.vector.tensor_tensor(f_top, ruC_top, B_i[0], ALU.subtract)
            ftb = wks.tile([128, n_el], BF16, tag="ftb")
            nc.vector.tensor_copy(ftb, f_top)
            fbb = wks.tile([128, n_el], BF16, tag="fbb")
            nc.vector.tensor_copy(fbb[0:32, :], f_bot)
            nc.vector.tensor_copy(fbb[32:64, :], fbb[0:32, :])
            nc.vector.tensor_copy(fbb[64:128, :], fbb[0:64, :])
            fbbD = wks.tile([128, n_el], BF16, tag="fbbD")
            nc.vector.memset(fbbD, 0.0)
            for a in range(4):
                nc.vector.tensor_copy(
                    _strided_cols(fbbD, a, 4, Q, part=(32 * a, 32 * a + 32)),
                    _strided_cols(fbb, a, 4, Q, part=(32 * a, 32 * a + 32)))

            xP = pspool.tile([128, n_el], F32, tag="ps")
            nc.tensor.matmul(xP, ident, S_all, start=True, stop=False,
                             skip_group_check=True)
            for n in range(n_el):
                nc.tensor.matmul(_col(xP, n), HallT[:, n * 128:(n + 1) * 128],
                                 _col(ftb, n),
                                 start=False, stop=False, skip_group_check=True)
            for q in range(Q):
                nc.tensor.matmul(xP[:, 4 * q:4 * q + 4],
                                 HallB[:, q * 128:(q + 1) * 128],
                                 fbbD[:, 4 * q:4 * q + 4],
                                 start=False, stop=(q == Q - 1),
                                 skip_group_check=True)
            nc.vector.tensor_copy(xo, xP)
            if taps:
                nc.sync.dma_start(out=dbg_d[7, :, 0:n_el], in_=s_i[0])
            xT = pspool.tile([n_el, 128], F32, tag="ps")
            nc.tensor.transpose(xT, xo, ident)
            nc.vector.tensor_copy(xout, xT)
            nc.sync.dma_start(out=out_d[0:n_el, :, 0], in_=xout)

    nc.compile()
    return nc


_NC_CACHE = {}


def _get_nc(taps=False):
    key = taps
    if key not in _NC_CACHE:
        _NC_CACHE[key] = build(taps=taps)
    return _NC_CACHE[key]


def run(inputs, taps=False, trace=False):
    nc = _get_nc(taps=taps)
    in_maps = []
    for c in range(NCORES):
        sl = slice(c * P, (c + 1) * P)
        in_maps.append({k: np.ascontiguousarray(np.asarray(v)[sl], dtype=np.float32)
                        for k, v in inputs.items()})
    res = run_bass_kernel_spmd(nc, in_maps, core_ids=list(range(NCORES)),
                               trace=trace)
    out = np.concatenate([res.results[c]["out"] for c in range(NCORES)], axis=0)
    return out, res


def kernel(**inputs):
    out, _ = run(inputs)
    return out

